# revision 46
# baseline (speedup 1.0000x reference)
"""Trainium2 Bass kernel for nn_MatrixSkipgram (embedding_lookup).

out[b] = ctx[X_context[b]] . (functor[X_functor[b]].reshape(E,E) @ noun[X_argument[b]])

Strategy (8 NeuronCores, functor-vocab sharded with dedup):
  - Dominant cost: streaming the unique functor rows (~5.6K of 10K vocab for
    8192 draws), 20KB each in f16 -> ~14MB/core.  The host groups batch
    elements by functor value and routes each unique functor to exactly one
    core (dedup), so every row is fetched once device-wide.
  - Transport: all big streams go through SWDGE (gpsimd dma_gather with
    identity indices) over 4 queues.  The staged blocks are contiguous in
    DRAM, so the software desc-gen coalesces rows into ~158KB descriptor
    chains that all 16 DMA engines pull at ~24.5 B/ns each (~390 GB/s
    per core), vs ~18.6 B/ns over <=10 engines for per-line HWDGE 2D DMAs.
  - Per core the rows are split between two compute engines so combined
    consumption (~23 rows/us) exceeds the DMA feed rate, keeping the kernel
    DMA-bound:
      * DVE set (384 single-use rows): 3 tiles of 128 rows, one row per SBUF
        partition, streamed as 6 half-tile chunks.  A fused prefix-scan
        (custom DVE op, f32 state) computes all 128 matvecs per tile; an
        Abel-summation STT against g[i] = ctx[i]-ctx[i+1] folds the
        segment-diff and the ctx dot product into one small op.
      * PE set (multi-use rows k>=2 plus leftover singles, ~320 rows): the
        host uploads M^T blocks packed [100, n*100]; the tensor engine runs
        one LDWEIGHTS+matmul per functor with that functor's k argument
        vectors as moving columns (LDWEIGHTS pipelines with the previous
        matmul) -> e-vectors accumulate across PSUM banks [100, <=512].
        Per bank: prodE = e * ctxT (DVE), then a ones-vector matmul reduces
        partitions -> out[1, cols].
  - Everything is SBUF-resident (~135KB/partition), no buffer recycling, so
    all transfers are issued up-front in consumption order and stream
    back-to-back.
  - The small noun/context lookups are resolved on the host (<1MB): per-slot
    argument vectors and g vectors are uploaded pre-gathered in f16.
  - Tables stream in TABLE_DT (default f16); all accumulation is f32
    (scan state, PSUM).
"""

import os
import sys

import numpy as np

if "/opt/trn_rl_repo" not in sys.path:
    sys.path.insert(0, "/opt/trn_rl_repo")

NOUN_VOCAB = 50000
FUNC_VOCAB = 10000
CTX_VOCAB = 50000
E = 100
ROW = E * E  # 10000
BATCH = 8192
N_CORES = 8
P = 128

TABLE_DT = os.environ.get("MSG_TABLE_DT", "f16")  # f16 | f32
DVE_TILES = int(os.environ.get("MSG_DVE_TILES", "3"))  # scan tiles of 128 rows
SPLIT = int(os.environ.get("MSG_SCAN_SPLIT", "4"))  # sub-scans per tile
KCAP = int(os.environ.get("MSG_KCAP", "6"))  # max moving cols per PE matmul
CHUNK = int(os.environ.get("MSG_CHUNK", "48"))  # functors per stationary gather
BANK = 512  # PSUM bank cols (f32)
NQ = 4  # SWDGE queues
IDX_DMA = os.environ.get("MSG_IDX_DMA", "1") == "1"  # DMA indices vs on-device iota
# gather:   dma_gather over 4 SWDGE queues (needs ~14us library load first,
#           which the HWDGE head covers) -- fastest measured configuration
# mainline: plain gpsimd.dma_start (no library load but a single SWDGE
#           queue; measured ~20us slower)
TRANSPORT = os.environ.get("MSG_TRANSPORT", "gather")

assert E % SPLIT == 0
SUB_W = ROW // SPLIT  # table cols per sub-scan chunk
_DTB = 2 if TABLE_DT == "f16" else 4


def _pad256(elems):
    """Round elems up so elems * dtype_bytes is a multiple of 256."""
    q = 256 // _DTB
    return (elems + q - 1) // q * q


SUBPAD = _pad256(SUB_W)

_cache = {}


def _register_mac_scan():
    """Custom DVE op: out[p,k] = cumsum_k(in0[p,k] * in1[p,k]) (f32 state)."""
    import concourse.dve_ops as dve_ops
    from concourse.dve_ops import OPS, DveOp
    from concourse.dve_spec import AluOp, Spec, Src0, Src1, _has_src1, lower, scan
    from concourse.dve_uop import DveOpSpec

    name = "MAC_SCAN_EMB"
    for o in OPS:
        if o.name == name:
            return o

    def _ref(in0, in1, s0, s1, imm2):
        p0 = in0.reshape(in0.shape[0], -1).astype(np.float32)
        p1 = np.broadcast_to(in1, in0.shape).reshape(in0.shape[0], -1).astype(np.float32)
        return np.cumsum(p0 * p1, axis=-1, dtype=np.float32).reshape(in0.shape)

    spec = Spec(body=scan(AluOp.ADD, Src0 * Src1), reference=_ref)
    row = max(dve_ops._SUB_OPCODE_FOR_NAME.values()) + 1
    assert row < 0x20
    shas = {}
    for ver in ("v3", "v4"):
        s = DveOpSpec(name=name, opcode=row, uops=lower(spec, ver=ver), rd1_en=_has_src1(spec))
        shas[ver] = s.sha(ver)
    dve_ops._SUB_OPCODE_FOR_NAME[name] = row
    op = DveOp(name, spec, subdim=False, uops_sha=shas)
    OPS.append(op)
    dve_ops.CUSTOM_DVE_SPECS[name] = spec
    return op


class _Plan:
    __slots__ = (
        "n_dve_slots", "n_pe_slots", "pe_class_counts", "pe_cols",
        "c_pe", "banks", "dve_rows", "dve_batch", "pe_rows", "pe_batch",
    )


def _build_plan(Xf):
    """Group batch elements by functor, route groups to cores (dedup), and
    build a uniform (SPMD) layout: DVE_TILES*128 single-use slots + per-k
    PE slots padded to the cross-core max (round-robin deal keeps padding
    to a few rows)."""
    n_dve = DVE_TILES * P

    order = np.argsort(Xf, kind="stable")
    vals, starts, counts = np.unique(Xf[order], return_index=True, return_counts=True)
    groups = [order[s : s + c] for s, c in zip(starts, counts)]

    # split big groups so every PE matmul has <= KCAP moving columns
    split = []
    for f, g in zip(vals, groups):
        for i in range(0, len(g), KCAP):
            split.append((int(f), g[i : i + KCAP]))

    multis = [t for t in split if len(t[1]) > 1]
    singles = [t for t in split if len(t[1]) == 1]

    # deal multis round-robin per k-class (k desc) -> per-class counts
    # differ by <=1 across cores
    multis.sort(key=lambda t: -len(t[1]))
    core_multis = [[] for _ in range(N_CORES)]
    for i, t in enumerate(multis):
        core_multis[i % N_CORES].append(t)

    # deal singles so per-core TOTAL rows are balanced
    total_rows = len(multis) + len(singles)
    base, extra = divmod(total_rows, N_CORES)
    targets = [base + (1 if c < extra else 0) for c in range(N_CORES)]
    core_singles = [[] for _ in range(N_CORES)]
    pos = 0
    for c in range(N_CORES):
        want = targets[c] - len(core_multis[c])
        assert want >= n_dve, (
            f"core {c}: only {want} singles for {n_dve} DVE slots"
        )
        core_singles[c] = singles[pos : pos + want]
        pos += want
    assert pos == len(singles)

    percore = []
    for c in range(N_CORES):
        dve = core_singles[c][:n_dve]
        pe = core_multis[c] + core_singles[c][n_dve:]
        percore.append((dve, pe))

    class_counts = np.zeros(KCAP + 1, dtype=np.int64)
    for _, pe in percore:
        cc = np.zeros(KCAP + 1, dtype=np.int64)
        for f, g in pe:
            cc[len(g)] += 1
        class_counts = np.maximum(class_counts, cc)

    plan = _Plan()
    plan.n_dve_slots = n_dve
    plan.pe_class_counts = [int(class_counts[k]) for k in range(KCAP + 1)]
    plan.n_pe_slots = int(sum(plan.pe_class_counts[1:]))

    # column layout: classes desc k; a matmul never straddles a PSUM bank
    pe_cols = []  # per PE slot: (col, k)
    banks = []  # (col_start, col_end)
    col = 0
    bank_start = 0
    for k in range(KCAP, 0, -1):
        for _ in range(plan.pe_class_counts[k]):
            if col + k - bank_start > BANK:
                banks.append((bank_start, col))
                bank_start = col
            pe_cols.append((col, k))
            col += k
    if col > bank_start:
        banks.append((bank_start, col))
    plan.pe_cols = pe_cols
    plan.c_pe = col
    plan.banks = banks

    plan.dve_rows = np.zeros((N_CORES, n_dve), dtype=np.int64)
    plan.dve_batch = np.full((N_CORES, n_dve), -1, dtype=np.int64)
    plan.pe_rows = np.full((N_CORES, plan.n_pe_slots), -1, dtype=np.int64)
    plan.pe_batch = np.full((N_CORES, plan.c_pe), -1, dtype=np.int64)
    for c, (dve, pe) in enumerate(percore):
        for s, (f, g) in enumerate(dve):
            plan.dve_rows[c, s] = f
            plan.dve_batch[c, s] = g[0]
        by_k = {}
        for f, g in pe:
            by_k.setdefault(len(g), []).append((f, g))
        r = 0
        for k in range(KCAP, 0, -1):
            got = by_k.get(k, [])
            assert len(got) <= plan.pe_class_counts[k]
            for i in range(plan.pe_class_counts[k]):
                col, kk = plan.pe_cols[r]
                assert kk == k
                if i < len(got):
                    f, g = got[i]
                    plan.pe_rows[c, r] = f
                    plan.pe_batch[c, col : col + len(g)] = g
                r += 1
    return plan


def _chunk_sizes(n):
    """PE stationary-stream chunk sizes: CHUNK rows each, with a small
    final chunk so the tail row lands (and computes) early."""
    sizes = []
    rem = n
    while rem > CHUNK + 8:
        sizes.append(CHUNK)
        rem -= CHUNK
    if rem > 8:
        sizes.append(rem - 8)
        rem = 8
    if rem:
        sizes.append(rem)
    return sizes


def _full_idx(fcols):
    """Host-side copy of the on-device index pattern: idx[p, c] = (p%16)+16c
    over [128, 8+fcols], with entries >= 100 in cols [0,8) patched to -1."""
    cols = 8 + fcols
    m = (np.arange(16)[:, None] + 16 * np.arange(cols)[None, :]).astype(np.int16)
    head = m[:, :8]
    head[head >= E] = -1
    return np.tile(m, (8, 1))


def _build(table_dt, n_dve_slots, n_pe_slots, c_pe, pe_cols, banks):
    import concourse.bacc as bacc
    import concourse.bass as bass
    import concourse.mybir as mybir
    from concourse.tile import TileContext

    f32 = mybir.dt.float32
    f16 = mybir.dt.float16
    i16 = mybir.dt.int16
    tdt = f32 if table_dt == "f32" else f16
    mult = mybir.AluOpType.mult

    mac_op = _register_mac_scan()

    n_tiles = n_dve_slots // P
    n_sub = n_tiles * SPLIT
    seg = E // SPLIT  # segments per sub-scan
    sizes = _chunk_sizes(n_pe_slots)
    wpads = [_pad256(w * E) for w in sizes]
    n_chunks = len(sizes)

    nc = bacc.Bacc(trn_type="TRN2", target_bir_lowering=False, debug=False,
                   num_swdge_queues=NQ, dynamic_dma_scratch_size=65536)

    # All staged tables lead with 128 dead rows so that a single on-device
    # iota (identity 0..127 per idx column-slice s, reading rows 128(s+1)+p)
    # serves every gather -- no input DMA is needed for the indices.
    scan_tab = nc.declare_dram_parameter(
        "scan_tab", [(n_sub + 1) * P, SUBPAD], tdt, isOutput=False
    )
    st_tabs = [
        nc.declare_dram_parameter(f"st_tab{i}", [E, wpads[i]], tdt, isOutput=False)
        for i in range(n_chunks)
    ]
    # combined small-input block: per partition p (row 128+p of the param):
    #   [ ag_p (n_tiles*2E) | arg_col_p (c_pe) | ctx_col_p (c_pe) | one ]
    agw = n_tiles * 2 * E
    inpw = _pad256(agw + 2 * c_pe + 1)
    inp_in = nc.declare_dram_parameter("inp_in", [2 * P, inpw], tdt, isOutput=False)
    out_dve = nc.declare_dram_parameter("out_dve", [P, n_sub], f32, isOutput=True)
    out_pe = nc.declare_dram_parameter("out_pe", [1, c_pe], f32, isOutput=True)
    fcols = n_sub * P // 16
    idx_in = (
        nc.declare_dram_parameter("idx_in", [P, 8 + fcols], i16, isOutput=False)
        if IDX_DMA and TRANSPORT != "mainline"
        else None
    )

    with TileContext(nc) as tc:
        with (
            tc.tile_pool(name="cpool", bufs=1) as cpool,
            tc.tile_pool(name="psum_e", bufs=2, space=bass.MemorySpace.PSUM) as psum_e,
            tc.tile_pool(name="psum_o", bufs=2, space=bass.MemorySpace.PSUM) as psum_o,
        ):
            # ---- persistent SBUF tiles (no recycling; everything fits) ----
            idx_t = cpool.tile([P, 8 + fcols], i16)
            inp_t = cpool.tile([P, 1, inpw], tdt)
            res = cpool.tile([P, n_sub], f32)
            pe_res = cpool.tile([1, c_pe], f32)
            scan_tiles = [cpool.tile([P, 1, SUBPAD], tdt, name=f"sc{s}") for s in range(n_sub)]
            st_tiles = [cpool.tile([P, 1, wpads[i]], tdt, name=f"st{i}") for i in range(n_chunks)]
            e_tiles = [cpool.tile([P, E], f32, name=f"e{t}") for t in range(n_tiles)]
            junk = cpool.tile([P, seg], f32)

            # views into the combined small-input block
            agc = inp_t[:, 0, 0:agw]
            argc = inp_t[0:E, 0, agw : agw + c_pe]
            ctxc = inp_t[0:E, 0, agw + c_pe : agw + 2 * c_pe]
            ones_t = inp_t[0:E, 0, agw + 2 * c_pe : agw + 2 * c_pe + 1]

            # ---- indices: idx[p, c] = (p%16) + 16c.
            # Columns [0,8) give identity 0..127 for the [100, *] gathers
            # (entries >= 100 patched to -1 = "ignore"); column-slice
            # [8(s+1), 8(s+2)) gives rows 128(s+1)+p, matching the 128 dead
            # rows leading each staged table. ----
            if TRANSPORT != "mainline":
                if IDX_DMA:
                    nc.sync.dma_start(out=idx_t[:], in_=idx_in[:])
                else:
                    for r in range(8):
                        nc.gpsimd.iota(
                            idx_t[r * 16 : (r + 1) * 16, :],
                            [[16, 8 + fcols]],
                            channel_multiplier=1,
                        )
                    for r in range(8):
                        nc.gpsimd.memset(idx_t[r * 16 + 4 : (r + 1) * 16, 6:7], -1)
                    nc.gpsimd.memset(idx_t[:, 7:8], -1)

            # ---- input streams ----
            if TRANSPORT == "mainline":
                # Everything through the mainline SWDGE queue (plain Pool
                # dma_start): no gather-library load (~14us), no indices,
                # contiguous staged blocks, issued in consumption order.
                def sdma(out, in_):
                    nc.gpsimd.dma_start(out=out, in_=in_)

                sdma(inp_t[:, 0, :], inp_in[P:, :])
                st_next = [0]

                def st_fetch(k):
                    for _ in range(k):
                        i = st_next[0]
                        if i < n_chunks:
                            sdma(st_tiles[i][0:E, 0, :], st_tabs[i][:])
                            st_next[0] += 1

                for s in range(n_sub):
                    sdma(
                        scan_tiles[s][:, 0, :],
                        scan_tab[(s + 1) * P : (s + 2) * P, :],
                    )
                    if s > 0:
                        st_fetch(1)
                st_fetch(n_chunks)
            else:
                # Hybrid: HWDGE head covers the ~14us gather-library load,
                # then dma_gather over 4 SWDGE queues.  One full tile's worth
                # of scan chunks rides HWDGE so the DVE never waits on the
                # late-starting SWDGE stream for its first tile.
                n_hw_scan = min(int(os.environ.get("MSG_HW_SCAN", str(SPLIT))), n_sub)
                hwq = [nc.sync, nc.scalar]
                hwq[0].dma_start(out=scan_tiles[0][:, 0, :], in_=scan_tab[P : 2 * P, :])
                hwq[1].dma_start(out=inp_t[:, 0, :], in_=inp_in[P:, :])
                for s in range(1, n_hw_scan):
                    hwq[s % 2].dma_start(
                        out=scan_tiles[s][:, 0, :],
                        in_=scan_tab[(s + 1) * P : (s + 2) * P, :],
                    )
                hwq[n_hw_scan % 2].dma_start(out=st_tiles[0][0:E, 0, :], in_=st_tabs[0][:])

                qi = [0]

                def gather(out_tile, src, idx_col, elem):
                    nc.gpsimd.dma_gather(
                        out_ap=out_tile[:],
                        in_ap=src[:],
                        idxs_ap=idx_t[:, idx_col : idx_col + 8],
                        num_idxs=P,
                        num_idxs_reg=P,
                        elem_size=elem,
                        queue_num=qi[0] % NQ,
                    )
                    qi[0] += 1

                st_next = [1]

                def st_fetch(k):
                    for _ in range(k):
                        i = st_next[0]
                        if i < n_chunks:
                            gather(st_tiles[i], st_tabs[i], 0, wpads[i])
                            st_next[0] += 1

                # two scan chunks (2 x 0.655MB) per stationary chunk
                # (0.94MB): scan bytes get ~58% of the stream, matching the
                # DVE's consumption rate, so the scans never starve while
                # the PE (which has more slack) absorbs the tail.
                emitted = 0
                for s in range(n_hw_scan, n_sub):
                    gather(scan_tiles[s], scan_tab, 8 + s * 8, SUBPAD)
                    emitted += 1
                    if emitted % 2 == 0:
                        st_fetch(1)
                st_fetch(n_chunks)

            # ---- PE emission (per chunk) ----
            chunk_r0 = np.cumsum([0] + sizes)
            pe_state = {"fill": 0, "cur": None}
            done_banks = []

            def emit_chunk(ch):
                r0, r1 = int(chunk_r0[ch]), int(chunk_r0[ch + 1])
                st = st_tiles[ch]
                for r in range(r0, r1):
                    col, k = pe_cols[r]
                    b0, b1 = banks[pe_state["fill"]]
                    if pe_state["cur"] is None:
                        pe_state["cur"] = psum_e.tile([E, BANK], f32, name="epsum", tag="epsum")
                    nc.tensor.matmul(
                        pe_state["cur"][:, col - b0 : col - b0 + k],
                        st[0:E, 0, (r - r0) * E : (r - r0 + 1) * E],
                        argc[:, col : col + k],
                        start=True,
                        stop=True,
                    )
                    if col + k == b1:
                        done_banks.append((pe_state["fill"], pe_state["cur"]))
                        pe_state["cur"] = None
                        pe_state["fill"] += 1

            def emit_flush(bi, ps):
                b0, b1 = banks[bi]
                w = b1 - b0
                prodE = cpool.tile([E, w], f16, name=f"prodE{bi}")
                nc.vector.tensor_tensor(
                    out=prodE[:], in0=ps[:, :w], in1=ctxc[:, b0:b1], op=mult
                )
                o_ps = psum_o.tile([1, BANK], f32, name="opsum", tag="opsum")
                nc.tensor.matmul(o_ps[0:1, :w], ones_t[:], prodE[:], start=True, stop=True)
                nc.scalar.copy(out=pe_res[0:1, b0:b1], in_=o_ps[0:1, :w])
                # per-bank output DMA: earlier banks' results ship while the
                # last bank is still flushing
                nc.scalar.dma_start(out=out_pe[0:1, b0:b1], in_=pe_res[0:1, b0:b1])

            # ---- DVE emission (per sub-scan) ----
            def emit_sub(s):
                t, sub = divmod(s, SPLIT)
                e_t = e_tiles[t]
                a = agc[:, t * 2 * E : t * 2 * E + E]
                g = agc[:, t * 2 * E + E : (t + 1) * 2 * E]
                i0 = sub * seg
                i1 = i0 + seg
                M3 = scan_tiles[s][:, 0, :SUB_W].rearrange("p (i j) -> p i j", j=E)
                argB = a.unsqueeze(1).broadcast_to([P, seg, E])
                eB = e_t[:, i0:i1].unsqueeze(2).broadcast_to([P, seg, E])
                nc.vector._custom_dve(mac_op, out=eB, in0=M3, in1=argB)
                nc.vector.scalar_tensor_tensor(
                    out=junk[:],
                    in0=e_t[:, i0:i1],
                    scalar=1.0,
                    in1=g[:, i0:i1],
                    op0=mult,
                    op1=mult,
                    accum_out=res[:, s : s + 1],
                )

            # ---- interleave compute emission ----
            # PSUM banks are complete well before the scans finish, so the
            # per-bank flushes slot into the DVE FIFO between the last
            # scans (the banks' matmuls are long done by then -- no stall)
            # instead of serializing after scan n_sub-1.
            flush_at = {n_sub - 1 - len(banks) + 1 + i: i for i in range(len(banks))}
            for i in range(max(n_chunks, n_sub)):
                if i < n_chunks:
                    emit_chunk(i)
                if i < n_sub:
                    if i in flush_at and flush_at[i] < len(done_banks):
                        bi, ps = done_banks[flush_at[i]]
                        emit_flush(bi, ps)
                    emit_sub(i)
            # any banks not flushed mid-stream (safety for small n_sub)
            flushed = {flush_at[i] for i in flush_at if flush_at[i] < len(done_banks)
                       and i < n_sub}
            for j, (bi, ps) in enumerate(done_banks):
                if j not in flushed:
                    emit_flush(bi, ps)
            assert pe_state["fill"] == len(banks)

            nc.sync.dma_start(out=out_dve[:], in_=res[:])
    nc.finalize()
    return nc


def _get_nc(plan):
    key = (
        TABLE_DT, plan.n_dve_slots, plan.n_pe_slots, plan.c_pe,
        tuple(plan.pe_class_counts), tuple(plan.banks), SPLIT, CHUNK,
        TRANSPORT, IDX_DMA, os.environ.get("MSG_HW_SCAN", ""),
    )
    if key not in _cache:
        _cache[key] = _build(
            TABLE_DT, plan.n_dve_slots, plan.n_pe_slots, plan.c_pe,
            plan.pe_cols, plan.banks,
        )
    return _cache[key]


def _prep_inputs(plan, Xa, Xf, Xc, noun, func, ctxt):
    tdt = np.float32 if TABLE_DT == "f32" else np.float16
    n_tiles = plan.n_dve_slots // P
    n_sub = n_tiles * SPLIT
    sizes = _chunk_sizes(plan.n_pe_slots)
    wpads = [_pad256(w * E) for w in sizes]
    chunk_r0 = np.cumsum([0] + sizes)
    in_maps = []
    seg = E // SPLIT
    agw = n_tiles * 2 * E
    c_pe = plan.c_pe
    inpw = _pad256(agw + 2 * c_pe + 1)

    for c in range(N_CORES):
        drows = plan.dve_rows[c]
        dbatch = plan.dve_batch[c]
        dve_rows_f = func[drows].astype(tdt)  # [n_dve, ROW]
        # pre-split into contiguous per-sub-scan blocks, 128 dead rows first
        scan_tab = np.zeros(((n_sub + 1) * P, SUBPAD), dtype=tdt)
        blk = dve_rows_f.reshape(n_tiles, P, SPLIT, SUB_W)
        for t in range(n_tiles):
            for h in range(SPLIT):
                s = t * SPLIT + h
                scan_tab[(s + 1) * P : (s + 2) * P, :SUB_W] = blk[t, :, h]

        dve_arg = noun[Xa[dbatch]].astype(tdt).reshape(n_tiles, P, E)
        ctx_rows = ctxt[Xc[dbatch]].astype(np.float32)
        g = np.empty_like(ctx_rows)
        for s in range(SPLIT):
            i0, i1 = s * seg, (s + 1) * seg
            g[:, i0 : i1 - 1] = ctx_rows[:, i0 : i1 - 1] - ctx_rows[:, i0 + 1 : i1]
            g[:, i1 - 1] = ctx_rows[:, i1 - 1]
        dve_g = g.astype(tdt).reshape(n_tiles, P, E)
        ag = np.concatenate([dve_arg, dve_g], axis=2)  # [n_tiles, P, 2E]
        dve_ag = ag.transpose(1, 0, 2).reshape(P, agw)

        prows = plan.pe_rows[c]
        psafe = np.where(prows >= 0, prows, 0)
        pe_statT = (
            func[psafe].reshape(-1, E, E).transpose(0, 2, 1)  # [r, j, i] = M_r[i, j]
            .transpose(1, 0, 2).reshape(E, -1).astype(tdt)    # [j, r*E + i]
        )
        pe_statT[:, (prows < 0).repeat(E)] = 0
        im = {}
        for i, (w, wp) in enumerate(zip(sizes, wpads)):
            r0, r1 = int(chunk_r0[i]), int(chunk_r0[i + 1])
            blk2 = np.zeros((E, wp), dtype=tdt)
            blk2[:, : w * E] = pe_statT[:, r0 * E : r1 * E]
            im[f"st_tab{i}"] = blk2
        pbatch = plan.pe_batch[c]
        pbsafe = np.where(pbatch >= 0, pbatch, 0)
        pe_arg = noun[Xa[pbsafe]].T.astype(tdt)  # [E, c_pe]
        pe_arg[:, pbatch < 0] = 0
        pe_ctx = ctxt[Xc[pbsafe]].T.astype(tdt)  # [E, c_pe]

        inp = np.zeros((2 * P, inpw), dtype=tdt)
        inp[P:, :agw] = dve_ag
        inp[P : P + E, agw : agw + c_pe] = pe_arg
        inp[P : P + E, agw + c_pe : agw + 2 * c_pe] = pe_ctx
        inp[P : P + E, agw + 2 * c_pe] = 1.0
        im.update({"scan_tab": scan_tab, "inp_in": inp})
        if IDX_DMA and TRANSPORT != "mainline":
            im["idx_in"] = _full_idx(n_sub * P // 16)
        in_maps.append(im)
    return in_maps


def run(inputs, trace=False, **kw):
    """Run the SPMD kernel; returns (full_output [8192] f32, BassKernelResults)."""
    from concourse.bass_utils import run_bass_kernel_spmd

    Xa = np.asarray(inputs["X_argument"], dtype=np.int64)
    Xf = np.asarray(inputs["X_functor"], dtype=np.int64)
    Xc = np.asarray(inputs["X_context"], dtype=np.int64)
    noun = np.asarray(inputs["noun_matrix"], dtype=np.float32)
    func = np.asarray(inputs["functor_table"], dtype=np.float32)
    ctxt = np.asarray(inputs["context_table"], dtype=np.float32)

    plan = _build_plan(Xf)
    nc = _get_nc(plan)
    in_maps = _prep_inputs(plan, Xa, Xf, Xc, noun, func, ctxt)
    r = run_bass_kernel_spmd(nc, in_maps, list(range(N_CORES)), trace=trace, **kw)

    out = np.zeros(BATCH, dtype=np.float32)
    n_tiles = plan.n_dve_slots // P
    for c in range(N_CORES):
        od = np.asarray(r.results[c]["out_dve"]).reshape(P, n_tiles, SPLIT).sum(-1)
        op = np.asarray(r.results[c]["out_pe"]).reshape(-1)  # [c_pe]
        dbatch = plan.dve_batch[c].reshape(n_tiles, P)
        for t in range(n_tiles):
            out[dbatch[t]] = od[:, t]
        sel = plan.pe_batch[c] >= 0
        out[plan.pe_batch[c][sel]] = op[sel]
    return out, r


def kernel(**inputs) -> np.ndarray:
    out, _ = run(inputs, trace=False)
    return out


if __name__ == "__main__":
    rng = np.random.default_rng(0)
    inputs = {
        "X_argument": rng.integers(0, NOUN_VOCAB, BATCH).astype(np.int32),
        "X_functor": rng.integers(0, FUNC_VOCAB, BATCH).astype(np.int32),
        "X_context": rng.integers(0, CTX_VOCAB, BATCH).astype(np.int32),
        "noun_matrix": rng.standard_normal((NOUN_VOCAB, E), dtype=np.float32),
        "functor_table": rng.standard_normal((FUNC_VOCAB, ROW), dtype=np.float32),
        "context_table": rng.standard_normal((CTX_VOCAB, E), dtype=np.float32),
    }
    out = kernel(**inputs)
    print(out.shape, out.dtype, out[:4])
